# revision 25
# baseline (speedup 1.0000x reference)
"""Linformer-style multihead attention on 8 Trainium2 NeuronCores.

Shapes (hardcoded): B=4, S=8192, D=512, H=8, DK=DV=64, PK=256.

Sharding: core c handles batch b=c//2, sequence half h=c%2 (4096 query rows).
The Linformer K/V projections contract over the FULL sequence, so each core
computes VP = We^T @ value[b] and VF = Wf^T @ value[b] over all 8192 rows
(redundant within a batch-pair, but avoids cross-core collectives).

Key algebra (reassociation): reference computes k = value@Wk then We^T@k.
We instead compute VP = We^T@value (8192-contraction) then kh = VP@Wk
(512-contraction), cutting total FLOPs from ~103G to ~71G. Biases fold in as
rank-1 augmentation rows of the small matmuls:
  kh = VP@Wk + outer(sum(We,0), bk) + outer(1, be)   (and same for vh).

On-chip pipeline is feature-major: query is transposed during DMA (bf16
x-bar transpose), projections produce qhT/khT with head dim on partitions,
scores come out [pk, s], softmax runs as exp (ACT) + denominator via an
appended ones-column on vh (so Z falls out of the AV matmul), reciprocal on
ACT, broadcast of 1/Z via a K=1 matmul, and the final Wo stage accumulates
all 8 heads into one PSUM tile in seq-major layout for contiguous output DMA.
"""

import numpy as np
import ml_dtypes
from contextlib import ExitStack

import concourse.bass as bass
import concourse.bacc as bacc
import concourse.mybir as mybir
import concourse.tile as tile
from concourse import bass_utils
from concourse.masks import make_identity

B, S, D = 4, 8192, 512
H, DK, DV, PK = 8, 64, 64, 256
SH = S // 2  # per-core query rows
NCORES = 8
P = 128

F32 = mybir.dt.float32
BF16 = mybir.dt.bfloat16
F32R = mybir.dt.float32r
AF = mybir.ActivationFunctionType
OP = mybir.AluOpType

_CACHE = {}


def _build_kernel(dbg=False):
    nc = bacc.Bacc(
        trn_type="TRN2",
        target_bir_lowering=False,
        debug=False,
        num_devices=NCORES,
    )

    q_t = nc.dram_tensor("q", [SH, D], BF16, kind="ExternalInput").ap()
    v_t = nc.dram_tensor("v", [S, D], BF16, kind="ExternalInput").ap()
    we_t = nc.dram_tensor("we", [S, PK], BF16, kind="ExternalInput").ap()
    wf_t = nc.dram_tensor("wf", [S, PK], BF16, kind="ExternalInput").ap()
    wq_t = nc.dram_tensor("wq", [D, D], BF16, kind="ExternalInput").ap()
    wk_t = nc.dram_tensor("wk", [D, D], BF16, kind="ExternalInput").ap()
    wv_t = nc.dram_tensor("wv", [D, D], BF16, kind="ExternalInput").ap()
    wo_t = nc.dram_tensor("wo", [D, D], BF16, kind="ExternalInput").ap()
    wkaug_t = nc.dram_tensor("wkaug", [2, D], BF16, kind="ExternalInput").ap()
    auge_t = nc.dram_tensor("auge", [2, PK], BF16, kind="ExternalInput").ap()
    wvaug_t = nc.dram_tensor("wvaug", [2, D], BF16, kind="ExternalInput").ap()
    augf_t = nc.dram_tensor("augf", [2, PK], BF16, kind="ExternalInput").ap()
    bq_t = nc.dram_tensor("bq", [D], F32, kind="ExternalInput").ap()
    bo_t = nc.dram_tensor("bo", [D], F32, kind="ExternalInput").ap()
    out_t = nc.dram_tensor("out", [SH, D], F32, kind="ExternalOutput").ap()

    if dbg:
        dbg_qT = nc.dram_tensor("dbg_qT", [P, 4, SH], BF16, kind="ExternalOutput").ap()
        dbg_qhT = nc.dram_tensor("dbg_qhT", [P, 4, SH], BF16, kind="ExternalOutput").ap()
        dbg_vp = nc.dram_tensor("dbg_vp", [P, 2, D], BF16, kind="ExternalOutput").ap()
        dbg_vpT = nc.dram_tensor("dbg_vpT", [P, 4, PK], BF16, kind="ExternalOutput").ap()
        dbg_khT = nc.dram_tensor("dbg_khT", [P, 4, PK], BF16, kind="ExternalOutput").ap()
        dbg_vh = nc.dram_tensor("dbg_vh", [P, 2, H, P], F32R, kind="ExternalOutput").ap()
        dbg_e = nc.dram_tensor("dbg_e", [P, 2, 512], F32R, kind="ExternalOutput").ap()
        dbg_av = nc.dram_tensor("dbg_av", [P, 4, 512], BF16, kind="ExternalOutput").ap()

    NT = SH // 512  # 8 s-tiles of 512

    with ExitStack() as ctx:
        tc = ctx.enter_context(tile.TileContext(nc))
        consts = ctx.enter_context(tc.tile_pool(name="consts", bufs=1))
        big = ctx.enter_context(tc.tile_pool(name="big", bufs=1))

        # ---- persistent activations (alloc first: DMA-transpose must be
        # the very first HWDGE traffic so xbar-mode serialization doesn't
        # pile sync-waits onto the transpose instruction) ----
        qTraw = big.tile([P, 4, SH], BF16)   # query transposed, feature-major
        for dc in range(4):
            nc.sync.dma_start(
                out=qTraw[:, dc, :],
                in_=q_t[:, dc * P:(dc + 1) * P],
                transpose=True,
            )

        # ---- constants / weights in SBUF ----
        wq_sb = consts.tile([P, 4, D], BF16)
        nc.gpsimd.dma_start(out=wq_sb, in_=wq_t.rearrange("(c p) e -> p c e", p=P))
        wk_sb = consts.tile([P, 4, D], BF16)
        nc.gpsimd.dma_start(out=wk_sb, in_=wk_t.rearrange("(c p) e -> p c e", p=P))
        wv_sb = consts.tile([P, 4, D], BF16)
        nc.gpsimd.dma_start(out=wv_sb, in_=wv_t.rearrange("(c p) e -> p c e", p=P))
        wo_sb = consts.tile([P, 4, D], BF16)
        nc.gpsimd.dma_start(out=wo_sb, in_=wo_t.rearrange("(c p) e -> p c e", p=P))
        wkaug_sb = consts.tile([2, D], BF16)
        nc.gpsimd.dma_start(out=wkaug_sb, in_=wkaug_t)
        auge_sb = consts.tile([2, PK], BF16)
        nc.gpsimd.dma_start(out=auge_sb, in_=auge_t)
        wvaug_sb = consts.tile([2, D], BF16)
        nc.gpsimd.dma_start(out=wvaug_sb, in_=wvaug_t)
        augf_sb = consts.tile([2, PK], BF16)
        nc.gpsimd.dma_start(out=augf_sb, in_=augf_t)
        bq_sb = consts.tile([P, 4], F32)
        nc.gpsimd.dma_start(out=bq_sb, in_=bq_t.rearrange("(c p) -> p c", p=P))
        bo_sb = consts.tile([P, D], F32)
        bo_bcast = bass.AP(tensor=bo_t.tensor, offset=bo_t.offset,
                           ap=[[0, P]] + list(bo_t.ap))
        nc.gpsimd.dma_start(out=bo_sb, in_=bo_bcast)
        ident_sb = consts.tile([P, P], BF16)
        make_identity(nc, ident_sb)

        # ---- persistent activations ----
        qhT = big.tile([P, 4, SH], BF16)     # q-proj, feature-major, scaled+bias
        khT = big.tile([P, 4, PK], BF16)     # [dk(2 heads/row-block), pair, pk]
        vh_sb = big.tile([P, 2, H, P], F32R)  # [pk rows, chunk, head, dv + 64 ones]
        vpT = big.tile([P, 4, PK], BF16)
        vfT = big.tile([P, 4, PK], BF16)
        vp_sb = big.tile([P, 2, D], BF16)
        vf_sb = big.tile([P, 2, D], BF16)

        vone_f32 = consts.tile([P, 2, H, 64], F32)
        nc.vector.memset(vone_f32, 1.0)
        nc.vector.tensor_copy(out=vh_sb[:, :, :, 64:], in_=vone_f32)

        # ---- phase B: VP = We^T @ value, VF = Wf^T @ value (full S) ----
        v_r = v_t.rearrange("(n p) d -> p n d", p=P)     # [128, 64, 512]
        we_r = we_t.rearrange("(n p) k -> p n k", p=P)   # [128, 64, 256]
        wf_r = wf_t.rearrange("(n p) k -> p n k", p=P)
        NSUP = 4
        CH = 64 // NSUP  # 16 chunks per super-chunk
        with (
            tc.tile_pool(name="vstream", bufs=2) as vstream,
            tc.tile_pool(name="wstream", bufs=2) as wstream,
            tc.tile_pool(name="accp", bufs=4, space="PSUM") as accp,
        ):
            vp_ps = [accp.tile([P, D], F32, tag="acc", name=f"vp_ps{i}")
                     for i in range(2)]
            vf_ps = [accp.tile([P, D], F32, tag="acc", name=f"vf_ps{i}")
                     for i in range(2)]
            for sc in range(NSUP):
                val_sb = vstream.tile([P, CH, D], BF16, tag="val")
                nc.gpsimd.dma_start(out=val_sb, in_=v_r[:, sc * CH:(sc + 1) * CH, :])
                we_sb = wstream.tile([P, CH, PK], BF16, tag="we")
                nc.gpsimd.dma_start(out=we_sb, in_=we_r[:, sc * CH:(sc + 1) * CH, :])
                wf_sb = wstream.tile([P, CH, PK], BF16, tag="wf")
                nc.gpsimd.dma_start(out=wf_sb, in_=wf_r[:, sc * CH:(sc + 1) * CH, :])
                for i in range(CH):
                    k = sc * CH + i
                    first, last = (k == 0), (k == 63)
                    for ps in range(2):
                        nc.tensor.matmul(
                            vp_ps[ps], lhsT=we_sb[:, i, ps * P:(ps + 1) * P],
                            rhs=val_sb[:, i, :], start=first, stop=last)
                        nc.tensor.matmul(
                            vf_ps[ps], lhsT=wf_sb[:, i, ps * P:(ps + 1) * P],
                            rhs=val_sb[:, i, :], start=first, stop=last)
            for ps in range(2):
                nc.vector.tensor_copy(out=vp_sb[:, ps, :], in_=vp_ps[ps])
                nc.vector.tensor_copy(out=vf_sb[:, ps, :], in_=vf_ps[ps])

        # transpose VP/VF to feature-major via PE (full-tile transpose)
        with tc.tile_pool(name="trp", bufs=4, space="PSUM") as trp:
            for ps in range(2):
                for eb in range(4):
                    tp = trp.tile([P, P], BF16, tag="tr", name=f"tp{ps}{eb}")
                    nc.tensor.transpose(
                        out=tp, in_=vp_sb[:, ps, eb * P:(eb + 1) * P],
                        identity=ident_sb)
                    nc.vector.tensor_copy(
                        out=vpT[:, eb, ps * P:(ps + 1) * P], in_=tp)
                    tf = trp.tile([P, P], BF16, tag="tr", name=f"tf{ps}{eb}")
                    nc.tensor.transpose(
                        out=tf, in_=vf_sb[:, ps, eb * P:(eb + 1) * P],
                        identity=ident_sb)
                    nc.vector.tensor_copy(
                        out=vfT[:, eb, ps * P:(ps + 1) * P], in_=tf)

        if dbg:
            nc.gpsimd.dma_start(out=dbg_qT, in_=qTraw)
            nc.gpsimd.dma_start(out=dbg_vp, in_=vp_sb)
            nc.gpsimd.dma_start(out=dbg_vpT, in_=vpT)

        # khT[e', pk] = Wk^T @ VPT + rank-1 bias rows
        with tc.tile_pool(name="khp", bufs=2, space="PSUM") as khp:
            for pr in range(4):
                ps_t = khp.tile([P, PK], F32, tag="kh")
                for ec in range(4):
                    nc.tensor.matmul(
                        ps_t, lhsT=wk_sb[:, ec, pr * P:(pr + 1) * P],
                        rhs=vpT[:, ec, :], start=(ec == 0), stop=False)
                nc.tensor.matmul(
                    ps_t, lhsT=wkaug_sb[:, pr * P:(pr + 1) * P],
                    rhs=auge_sb, start=False, stop=True)
                nc.vector.tensor_copy(out=khT[:, pr, :], in_=ps_t)

        # vh[pk, dv(+1)] = VFT^T @ Wv + rank-1 bias rows (seq-major in pk)
        with tc.tile_pool(name="vhp", bufs=2, space="PSUM") as vhp:
            for ps in range(2):
                ps_t = vhp.tile([P, D], F32, tag="vh")
                for ec in range(4):
                    nc.tensor.matmul(
                        ps_t, lhsT=vfT[:, ec, ps * P:(ps + 1) * P],
                        rhs=wv_sb[:, ec, :], start=(ec == 0), stop=False)
                nc.tensor.matmul(
                    ps_t, lhsT=augf_sb[:, ps * P:(ps + 1) * P],
                    rhs=wvaug_sb, start=False, stop=True)
                nc.vector.tensor_copy(
                    out=vh_sb[:, ps, :, 0:64],
                    in_=ps_t.rearrange("p (h v) -> p h v", h=H))

        # ---- phase C: qhT = (Wq^T @ queryT) + bq (feature-major) ----
        with tc.tile_pool(name="qp", bufs=8, space="PSUM") as qp:
            for eb in range(4):
                ps_ts = [qp.tile([P, 512], F32, tag="q", name=f"qps{eb}_{st}")
                         for st in range(NT)]
                for dc in range(4):
                    for st in range(NT):
                        nc.tensor.matmul(
                            ps_ts[st], lhsT=wq_sb[:, dc, eb * P:(eb + 1) * P],
                            rhs=qTraw[:, dc, st * 512:(st + 1) * 512],
                            start=(dc == 0), stop=(dc == 3))
                for st in range(NT):
                    nc.vector.tensor_scalar(
                        out=qhT[:, eb, st * 512:(st + 1) * 512],
                        in0=ps_ts[st], scalar1=bq_sb[:, eb:eb + 1], scalar2=None,
                        op0=OP.add)

        if dbg:
            nc.gpsimd.dma_start(out=dbg_qhT, in_=qhT)
            nc.gpsimd.dma_start(out=dbg_khT, in_=khT)
            nc.gpsimd.dma_start(out=dbg_vh, in_=vh_sb)

        # ---- phase D: attention + output projection ----
        out_r = out_t.rearrange("(t c p) d -> t p c d", c=4, p=P)
        with (
            tc.tile_pool(name="scp", bufs=2, space="PSUM") as scp,
            tc.tile_pool(name="nump", bufs=2, space="PSUM") as nump,
            tc.tile_pool(name="outp", bufs=2, space="PSUM") as outp,
            tc.tile_pool(name="epool", bufs=3) as epool,
            tc.tile_pool(name="rzp", bufs=4) as rzp,
            tc.tile_pool(name="avp", bufs=2) as avp,
            tc.tile_pool(name="ostage", bufs=2) as ostage,
        ):
            for st in range(NT):
                ssl = slice(st * 512, (st + 1) * 512)
                av_sb = avp.tile([P, 4, 512], BF16, tag="av")
                for h in range(H):
                    pr, hb = h // 2, (h % 2) * 64
                    e_sb = epool.tile([P, 2, 512], F32R, tag="e")
                    for ps in range(2):
                        sc_t = scp.tile([P, 512], F32, tag="sc")
                        nc.tensor.matmul(
                            sc_t,
                            lhsT=khT[hb:hb + 64, pr, ps * P:(ps + 1) * P],
                            rhs=qhT[hb:hb + 64, pr, ssl],
                            start=True, stop=True)
                        nc.scalar.activation(
                            out=e_sb[:, ps, :], in_=sc_t, func=AF.Exp)
                    n_t = nump.tile([P, 512], F32, tag="num")
                    for c in range(2):
                        nc.tensor.matmul(
                            n_t,
                            lhsT=vh_sb[:, c, h, :],
                            rhs=e_sb[:, c, :],
                            start=(c == 0), stop=(c == 1))
                    rzb = rzp.tile([64, 512], F32, tag="rzb")
                    nc.vector.reciprocal(out=rzb, in_=n_t[64:P, :])
                    nc.vector.tensor_tensor(
                        out=av_sb[hb:hb + 64, pr, :],
                        in0=n_t[0:64, :], in1=rzb, op=OP.mult)
                    if dbg and st == 0 and h == 0:
                        nc.gpsimd.dma_start(out=dbg_e, in_=e_sb)
                if dbg and st == 0:
                    nc.gpsimd.dma_start(out=dbg_av, in_=av_sb)
                o_sb = ostage.tile([P, 4, D], F32, tag="ost")
                for sl in range(4):
                    o_t = outp.tile([P, D], F32, tag="o")
                    for pr in range(4):
                        nc.tensor.matmul(
                            o_t, lhsT=av_sb[:, pr, sl * P:(sl + 1) * P],
                            rhs=wo_sb[:, pr, :], start=(pr == 0), stop=(pr == 3))
                    nc.vector.tensor_tensor(
                        out=o_sb[:, sl, :], in0=o_t, in1=bo_sb, op=OP.add)
                nc.sync.dma_start(out=out_r[st], in_=o_sb)

    nc.finalize()
    return nc


def _prep_inputs(inputs):
    bf = ml_dtypes.bfloat16
    f32 = np.float32
    q = np.ascontiguousarray(inputs["query"])
    v = np.ascontiguousarray(inputs["value"])
    We, Wf = np.asarray(inputs["We"]), np.asarray(inputs["Wf"])
    scale = np.float32(DK ** -0.5)
    ones = np.ones(D, f32)
    sWe = We.astype(f32).sum(0)
    sWf = Wf.astype(f32).sum(0)
    shared = {
        "we": We.astype(bf),
        "wf": Wf.astype(bf),
        "wq": (np.asarray(inputs["Wq"]) * scale).astype(bf),
        "wk": np.asarray(inputs["Wk"]).astype(bf),
        "wv": np.asarray(inputs["Wv"]).astype(bf),
        "wo": np.asarray(inputs["Wo"]).astype(bf),
        "wkaug": np.stack([np.asarray(inputs["bk"], f32), ones]).astype(bf),
        "auge": np.stack([sWe, np.asarray(inputs["be"], f32)]).astype(bf),
        "wvaug": np.stack([np.asarray(inputs["bv"], f32), ones]).astype(bf),
        "augf": np.stack([sWf, np.asarray(inputs["bf"], f32)]).astype(bf),
        "bq": (np.asarray(inputs["bq"]) * scale).astype(f32),
        "bo": np.asarray(inputs["bo"]).astype(f32),
    }
    in_maps = []
    for c in range(NCORES):
        b, half = c // 2, c % 2
        m = dict(shared)
        m["q"] = np.ascontiguousarray(q[b, half * SH:(half + 1) * SH, :]).astype(bf)
        m["v"] = np.ascontiguousarray(v[b]).astype(bf)
        in_maps.append(m)
    return in_maps


def kernel(**inputs):
    if "nc" not in _CACHE:
        _CACHE["nc"] = _build_kernel()
    nc = _CACHE["nc"]
    in_maps = _prep_inputs(inputs)
    res = bass_utils.run_bass_kernel_spmd(nc, in_maps, core_ids=list(range(NCORES)))
    out = np.empty((B, S, D), np.float32)
    for c in range(NCORES):
        b, half = c // 2, c % 2
        out[b, half * SH:(half + 1) * SH, :] = res.results[c]["out"]
    return out


# revision 27
# speedup vs baseline: 1.6725x; 1.6725x over previous
"""Linformer-style multihead attention on 8 Trainium2 NeuronCores.

Shapes (hardcoded): B=4, S=8192, D=512, H=8, DK=DV=64, PK=256.

Sharding: core c handles batch b=c//2, sequence half h=c%2 (4096 query rows).
The Linformer K/V projections contract over the FULL sequence, so each core
computes VP = We^T @ value[b] and VF = Wf^T @ value[b] over all 8192 rows
(redundant within a batch-pair, but avoids cross-core collectives).

Key algebra (reassociation): reference computes k = value@Wk then We^T@k.
We instead compute VP = We^T@value (8192-contraction) then kh = VP@Wk
(512-contraction), cutting total FLOPs from ~103G to ~71G. Biases fold in as
rank-1 augmentation rows of the small matmuls:
  kh = VP@Wk + outer(sum(We,0), bk) + outer(1, be)   (and same for vh).

On-chip pipeline is feature-major: query is transposed during DMA (bf16
x-bar transpose), projections produce qhT/khT with head dim on partitions,
scores come out [pk, s], softmax runs as exp (ACT) + denominator via an
appended ones-column on vh (so Z falls out of the AV matmul), reciprocal on
ACT, broadcast of 1/Z via a K=1 matmul, and the final Wo stage accumulates
all 8 heads into one PSUM tile in seq-major layout for contiguous output DMA.
"""

import numpy as np
import ml_dtypes
from contextlib import ExitStack

import concourse.bass as bass
import concourse.bacc as bacc
import concourse.mybir as mybir
import concourse.tile as tile
from concourse import bass_utils
from concourse.masks import make_identity

B, S, D = 4, 8192, 512
H, DK, DV, PK = 8, 64, 64, 256
SH = S // 2  # per-core query rows
NCORES = 8
P = 128

F32 = mybir.dt.float32
BF16 = mybir.dt.bfloat16
F32R = mybir.dt.float32r
AF = mybir.ActivationFunctionType
OP = mybir.AluOpType

_CACHE = {}


def _build_kernel(dbg=False):
    nc = bacc.Bacc(
        trn_type="TRN2",
        target_bir_lowering=False,
        debug=False,
        num_devices=NCORES,
    )

    q_t = nc.dram_tensor("q", [SH, D], BF16, kind="ExternalInput").ap()
    v_t = nc.dram_tensor("v", [S, D], BF16, kind="ExternalInput").ap()
    we_t = nc.dram_tensor("we", [S, PK], BF16, kind="ExternalInput").ap()
    wf_t = nc.dram_tensor("wf", [S, PK], BF16, kind="ExternalInput").ap()
    wq_t = nc.dram_tensor("wq", [D, D], BF16, kind="ExternalInput").ap()
    wk_t = nc.dram_tensor("wk", [D, D], BF16, kind="ExternalInput").ap()
    wv_t = nc.dram_tensor("wv", [D, D], BF16, kind="ExternalInput").ap()
    wo_t = nc.dram_tensor("wo", [D, D], BF16, kind="ExternalInput").ap()
    wkaug_t = nc.dram_tensor("wkaug", [2, D], BF16, kind="ExternalInput").ap()
    auge_t = nc.dram_tensor("auge", [2, PK], BF16, kind="ExternalInput").ap()
    wvaug_t = nc.dram_tensor("wvaug", [2, D], BF16, kind="ExternalInput").ap()
    augf_t = nc.dram_tensor("augf", [2, PK], BF16, kind="ExternalInput").ap()
    bq_t = nc.dram_tensor("bq", [D], F32, kind="ExternalInput").ap()
    bo_t = nc.dram_tensor("bo", [D], F32, kind="ExternalInput").ap()
    out_t = nc.dram_tensor("out", [SH, D], F32, kind="ExternalOutput").ap()

    if dbg:
        dbg_qT = nc.dram_tensor("dbg_qT", [P, 4, SH], BF16, kind="ExternalOutput").ap()
        dbg_qhT = nc.dram_tensor("dbg_qhT", [P, 4, SH], BF16, kind="ExternalOutput").ap()
        dbg_vp = nc.dram_tensor("dbg_vp", [P, 2, D], BF16, kind="ExternalOutput").ap()
        dbg_vpT = nc.dram_tensor("dbg_vpT", [P, 4, PK], BF16, kind="ExternalOutput").ap()
        dbg_khT = nc.dram_tensor("dbg_khT", [P, 4, PK], BF16, kind="ExternalOutput").ap()
        dbg_vh = nc.dram_tensor("dbg_vh", [P, 2, H, P], F32R, kind="ExternalOutput").ap()
        dbg_e = nc.dram_tensor("dbg_e", [P, 2, 512], F32R, kind="ExternalOutput").ap()
        dbg_av = nc.dram_tensor("dbg_av", [P, 4, 512], BF16, kind="ExternalOutput").ap()

    NT = SH // 512  # 8 s-tiles of 512

    with ExitStack() as ctx:
        tc = ctx.enter_context(tile.TileContext(nc))
        consts = ctx.enter_context(tc.tile_pool(name="consts", bufs=1))
        big = ctx.enter_context(tc.tile_pool(name="big", bufs=1))

        # ---- persistent activations (alloc first: DMA-transpose must be
        # the very first HWDGE traffic so xbar-mode serialization doesn't
        # pile sync-waits onto the transpose instruction) ----
        qTraw = big.tile([P, 4, SH], BF16)   # query transposed, feature-major
        for dc in range(4):
            nc.sync.dma_start(
                out=qTraw[:, dc, :],
                in_=q_t[:, dc * P:(dc + 1) * P],
                transpose=True,
            )

        # ---- constants / weights in SBUF ----
        wq_sb = consts.tile([P, 4, D], BF16)
        nc.gpsimd.dma_start(out=wq_sb, in_=wq_t.rearrange("(c p) e -> p c e", p=P))
        wk_sb = consts.tile([P, 4, D], BF16)
        nc.gpsimd.dma_start(out=wk_sb, in_=wk_t.rearrange("(c p) e -> p c e", p=P))
        wv_sb = consts.tile([P, 4, D], BF16)
        nc.gpsimd.dma_start(out=wv_sb, in_=wv_t.rearrange("(c p) e -> p c e", p=P))
        wo_sb = consts.tile([P, 4, D], BF16)
        nc.gpsimd.dma_start(out=wo_sb, in_=wo_t.rearrange("(c p) e -> p c e", p=P))
        wkaug_sb = consts.tile([2, D], BF16)
        nc.gpsimd.dma_start(out=wkaug_sb, in_=wkaug_t)
        auge_sb = consts.tile([2, PK], BF16)
        nc.gpsimd.dma_start(out=auge_sb, in_=auge_t)
        wvaug_sb = consts.tile([2, D], BF16)
        nc.gpsimd.dma_start(out=wvaug_sb, in_=wvaug_t)
        augf_sb = consts.tile([2, PK], BF16)
        nc.gpsimd.dma_start(out=augf_sb, in_=augf_t)
        bq_sb = consts.tile([P, 4], F32)
        nc.gpsimd.dma_start(out=bq_sb, in_=bq_t.rearrange("(c p) -> p c", p=P))
        bo_sb = consts.tile([P, D], F32)
        bo_bcast = bass.AP(tensor=bo_t.tensor, offset=bo_t.offset,
                           ap=[[0, P]] + list(bo_t.ap))
        nc.gpsimd.dma_start(out=bo_sb, in_=bo_bcast)
        ident_sb = consts.tile([P, P], BF16)
        make_identity(nc, ident_sb)

        # ---- persistent activations ----
        qhT = big.tile([P, 4, SH], BF16)     # q-proj, feature-major, scaled+bias
        khT = big.tile([P, 4, PK], BF16)     # [dk(2 heads/row-block), pair, pk]
        vh_sb = big.tile([P, 2, H, P], F32R)  # [pk rows, chunk, head, dv + 64 ones]
        vpT = big.tile([P, 4, PK], BF16)
        vfT = big.tile([P, 4, PK], BF16)
        vp_sb = big.tile([P, 2, D], BF16)
        vf_sb = big.tile([P, 2, D], BF16)

        vone_f32 = consts.tile([P, 2, H, 64], F32)
        nc.vector.memset(vone_f32, 1.0)
        nc.vector.tensor_copy(out=vh_sb[:, :, :, 64:], in_=vone_f32)

        # ---- phase B: VP = We^T @ value, VF = Wf^T @ value (full S) ----
        v_r = v_t.rearrange("(n p) d -> p n d", p=P)     # [128, 64, 512]
        we_r = we_t.rearrange("(n p) k -> p n k", p=P)   # [128, 64, 256]
        wf_r = wf_t.rearrange("(n p) k -> p n k", p=P)
        NSUP = 4
        CH = 64 // NSUP  # 16 chunks per super-chunk
        with (
            tc.tile_pool(name="vstream", bufs=2) as vstream,
            tc.tile_pool(name="wstream", bufs=2) as wstream,
            tc.tile_pool(name="accp", bufs=4, space="PSUM") as accp,
        ):
            vp_ps = [accp.tile([P, D], F32, tag="acc", name=f"vp_ps{i}")
                     for i in range(2)]
            vf_ps = [accp.tile([P, D], F32, tag="acc", name=f"vf_ps{i}")
                     for i in range(2)]
            for sc in range(NSUP):
                val_sb = vstream.tile([P, CH, D], BF16, tag="val")
                nc.gpsimd.dma_start(out=val_sb, in_=v_r[:, sc * CH:(sc + 1) * CH, :])
                we_sb = wstream.tile([P, CH, PK], BF16, tag="we")
                nc.gpsimd.dma_start(out=we_sb, in_=we_r[:, sc * CH:(sc + 1) * CH, :])
                wf_sb = wstream.tile([P, CH, PK], BF16, tag="wf")
                nc.gpsimd.dma_start(out=wf_sb, in_=wf_r[:, sc * CH:(sc + 1) * CH, :])
                for i in range(CH):
                    k = sc * CH + i
                    first, last = (k == 0), (k == 63)
                    for ps in range(2):
                        nc.tensor.matmul(
                            vp_ps[ps], lhsT=we_sb[:, i, ps * P:(ps + 1) * P],
                            rhs=val_sb[:, i, :], start=first, stop=last)
                        nc.tensor.matmul(
                            vf_ps[ps], lhsT=wf_sb[:, i, ps * P:(ps + 1) * P],
                            rhs=val_sb[:, i, :], start=first, stop=last)
            for ps in range(2):
                nc.vector.tensor_copy(out=vp_sb[:, ps, :], in_=vp_ps[ps])
                nc.vector.tensor_copy(out=vf_sb[:, ps, :], in_=vf_ps[ps])

        # transpose VP/VF to feature-major via PE (full-tile transpose)
        with tc.tile_pool(name="trp", bufs=4, space="PSUM") as trp:
            for ps in range(2):
                for eb in range(4):
                    tp = trp.tile([P, P], BF16, tag="tr", name=f"tp{ps}{eb}")
                    nc.tensor.transpose(
                        out=tp, in_=vp_sb[:, ps, eb * P:(eb + 1) * P],
                        identity=ident_sb)
                    nc.vector.tensor_copy(
                        out=vpT[:, eb, ps * P:(ps + 1) * P], in_=tp)
                    tf = trp.tile([P, P], BF16, tag="tr", name=f"tf{ps}{eb}")
                    nc.tensor.transpose(
                        out=tf, in_=vf_sb[:, ps, eb * P:(eb + 1) * P],
                        identity=ident_sb)
                    nc.vector.tensor_copy(
                        out=vfT[:, eb, ps * P:(ps + 1) * P], in_=tf)

        if dbg:
            nc.gpsimd.dma_start(out=dbg_qT, in_=qTraw)
            nc.gpsimd.dma_start(out=dbg_vp, in_=vp_sb)
            nc.gpsimd.dma_start(out=dbg_vpT, in_=vpT)

        # khT[e', pk] = Wk^T @ VPT + rank-1 bias rows
        with tc.tile_pool(name="khp", bufs=2, space="PSUM") as khp:
            for pr in range(4):
                ps_t = khp.tile([P, PK], F32, tag="kh")
                for ec in range(4):
                    nc.tensor.matmul(
                        ps_t, lhsT=wk_sb[:, ec, pr * P:(pr + 1) * P],
                        rhs=vpT[:, ec, :], start=(ec == 0), stop=False)
                nc.tensor.matmul(
                    ps_t, lhsT=wkaug_sb[:, pr * P:(pr + 1) * P],
                    rhs=auge_sb, start=False, stop=True)
                nc.vector.tensor_copy(out=khT[:, pr, :], in_=ps_t)

        # vh[pk, dv(+1)] = VFT^T @ Wv + rank-1 bias rows (seq-major in pk)
        with tc.tile_pool(name="vhp", bufs=2, space="PSUM") as vhp:
            for ps in range(2):
                ps_t = vhp.tile([P, D], F32, tag="vh")
                for ec in range(4):
                    nc.tensor.matmul(
                        ps_t, lhsT=vfT[:, ec, ps * P:(ps + 1) * P],
                        rhs=wv_sb[:, ec, :], start=(ec == 0), stop=False)
                nc.tensor.matmul(
                    ps_t, lhsT=augf_sb[:, ps * P:(ps + 1) * P],
                    rhs=wvaug_sb, start=False, stop=True)
                nc.vector.tensor_copy(
                    out=vh_sb[:, ps, :, 0:64],
                    in_=ps_t.rearrange("p (h v) -> p h v", h=H))

        # ---- phase C: qhT = (Wq^T @ queryT) + bq (feature-major) ----
        with tc.tile_pool(name="qp", bufs=8, space="PSUM") as qp:
            for eb in range(4):
                ps_ts = [qp.tile([P, 512], F32, tag="q", name=f"qps{eb}_{st}")
                         for st in range(NT)]
                for dc in range(4):
                    for st in range(NT):
                        nc.tensor.matmul(
                            ps_ts[st], lhsT=wq_sb[:, dc, eb * P:(eb + 1) * P],
                            rhs=qTraw[:, dc, st * 512:(st + 1) * 512],
                            start=(dc == 0), stop=(dc == 3))
                for st in range(NT):
                    nc.vector.tensor_scalar(
                        out=qhT[:, eb, st * 512:(st + 1) * 512],
                        in0=ps_ts[st], scalar1=bq_sb[:, eb:eb + 1], scalar2=None,
                        op0=OP.add)

        if dbg:
            nc.gpsimd.dma_start(out=dbg_qhT, in_=qhT)
            nc.gpsimd.dma_start(out=dbg_khT, in_=khT)
            nc.gpsimd.dma_start(out=dbg_vh, in_=vh_sb)

        # ---- phase D: attention + output projection ----
        out_r = out_t.rearrange("(t c p) d -> t p c d", c=4, p=P)
        with (
            tc.tile_pool(name="scp", bufs=3, space="PSUM") as scp,
            tc.tile_pool(name="nump", bufs=3, space="PSUM") as nump,
            tc.tile_pool(name="outp", bufs=2, space="PSUM") as outp,
            tc.tile_pool(name="epool", bufs=3) as epool,
            tc.tile_pool(name="rzp", bufs=4) as rzp,
            tc.tile_pool(name="avp", bufs=2) as avp,
            tc.tile_pool(name="ostage", bufs=2) as ostage,
        ):
            for st in range(NT):
                ssl = slice(st * 512, (st + 1) * 512)
                av_sb = avp.tile([P, 4, 512], BF16, tag="av")
                for h in range(H):
                    pr, hb = h // 2, (h % 2) * 64
                    e_sb = epool.tile([P, 2, 512], F32R, tag="e")
                    for ps in range(2):
                        sc_t = scp.tile([P, 512], F32, tag="sc")
                        nc.tensor.matmul(
                            sc_t,
                            lhsT=khT[hb:hb + 64, pr, ps * P:(ps + 1) * P],
                            rhs=qhT[hb:hb + 64, pr, ssl],
                            start=True, stop=True)
                        nc.scalar.activation(
                            out=e_sb[:, ps, :], in_=sc_t, func=AF.Exp)
                    n_t = nump.tile([P, 512], F32, tag="num")
                    for c in range(2):
                        nc.tensor.matmul(
                            n_t,
                            lhsT=vh_sb[:, c, h, :],
                            rhs=e_sb[:, c, :],
                            start=(c == 0), stop=(c == 1))
                    zz = rzp.tile([64, 512], F32, tag="zz")
                    nc.scalar.activation(out=zz, in_=n_t[64:P, :], func=AF.Copy)
                    rzb = rzp.tile([64, 512], F32, tag="rzb")
                    nc.vector.reciprocal_approx_fast(out=rzb, in_=zz)
                    nc.vector.tensor_tensor(
                        out=av_sb[hb:hb + 64, pr, :],
                        in0=n_t[0:64, :], in1=rzb, op=OP.mult)
                    if dbg and st == 0 and h == 0:
                        nc.gpsimd.dma_start(out=dbg_e, in_=e_sb)
                if dbg and st == 0:
                    nc.gpsimd.dma_start(out=dbg_av, in_=av_sb)
                o_sb = ostage.tile([P, 4, D], F32, tag="ost")
                for sl in range(4):
                    o_t = outp.tile([P, D], F32, tag="o")
                    for pr in range(4):
                        nc.tensor.matmul(
                            o_t, lhsT=av_sb[:, pr, sl * P:(sl + 1) * P],
                            rhs=wo_sb[:, pr, :], start=(pr == 0), stop=(pr == 3))
                    nc.vector.tensor_tensor(
                        out=o_sb[:, sl, :], in0=o_t, in1=bo_sb, op=OP.add)
                nc.sync.dma_start(out=out_r[st], in_=o_sb)

    nc.finalize()
    return nc


def _prep_inputs(inputs):
    bf = ml_dtypes.bfloat16
    f32 = np.float32
    q = np.ascontiguousarray(inputs["query"])
    v = np.ascontiguousarray(inputs["value"])
    We, Wf = np.asarray(inputs["We"]), np.asarray(inputs["Wf"])
    scale = np.float32(DK ** -0.5)
    ones = np.ones(D, f32)
    sWe = We.astype(f32).sum(0)
    sWf = Wf.astype(f32).sum(0)
    shared = {
        "we": We.astype(bf),
        "wf": Wf.astype(bf),
        "wq": (np.asarray(inputs["Wq"]) * scale).astype(bf),
        "wk": np.asarray(inputs["Wk"]).astype(bf),
        "wv": np.asarray(inputs["Wv"]).astype(bf),
        "wo": np.asarray(inputs["Wo"]).astype(bf),
        "wkaug": np.stack([np.asarray(inputs["bk"], f32), ones]).astype(bf),
        "auge": np.stack([sWe, np.asarray(inputs["be"], f32)]).astype(bf),
        "wvaug": np.stack([np.asarray(inputs["bv"], f32), ones]).astype(bf),
        "augf": np.stack([sWf, np.asarray(inputs["bf"], f32)]).astype(bf),
        "bq": (np.asarray(inputs["bq"]) * scale).astype(f32),
        "bo": np.asarray(inputs["bo"]).astype(f32),
    }
    in_maps = []
    for c in range(NCORES):
        b, half = c // 2, c % 2
        m = dict(shared)
        m["q"] = np.ascontiguousarray(q[b, half * SH:(half + 1) * SH, :]).astype(bf)
        m["v"] = np.ascontiguousarray(v[b]).astype(bf)
        in_maps.append(m)
    return in_maps


def kernel(**inputs):
    if "nc" not in _CACHE:
        _CACHE["nc"] = _build_kernel()
    nc = _CACHE["nc"]
    in_maps = _prep_inputs(inputs)
    res = bass_utils.run_bass_kernel_spmd(nc, in_maps, core_ids=list(range(NCORES)))
    out = np.empty((B, S, D), np.float32)
    for c in range(NCORES):
        b, half = c // 2, c % 2
        out[b, half * SH:(half + 1) * SH, :] = res.results[c]["out"]
    return out
